# revision 40
# baseline (speedup 1.0000x reference)
"""ContrastiveMagnitudeLoss on 8 Trainium2 NeuronCores (Bass/Tile).

Strategy (sharding_hint: shard batch across cores, all-gather target):
  - B=4096 rows of `predicted` are sharded 512/core. Every core gets the
    full (transposed) `target`, so each core owns complete rows of the
    B x B distance matrix and the row-softmax needs no communication.
  - The Gram identity  d^2[m,n] = ||p_m||^2 + ||t_n||^2 - 2 p_m.t_n  is
    computed entirely on the PE array by extending the contraction dim:
    4 extra K-rows carry (1, -tsq/2) and (-psq/2, 1) rank-1 terms (each
    split hi/lo in bf16 to keep f32-level accuracy), so PSUM directly
    holds X = -d^2/2.
  - ScalarE evaluates d = exp(0.5*ln(-2X)) (Ln+Exp share one ACT table
    set; Sqrt would force table thrashing and has a loose ULP budget),
    then exp(-10*d + b_i) with per-row bias b_i = 10*d_ii - 40 and a
    fused free-dim accumulation (accum_out) giving the softmax sums S_i.
    Algebra: logsumexp_i - logit_ii == ln(S_i) + 40 exactly, so only
    S_i [B] leaves the device for the contrastive term.
  - The magnitude-loss numerator sum_d |p - t| is reduced over the
    contraction dim with a ones-vector matmul on PE.
  - Host does the O(B*D) input prep (transpose/shard/row stats) and the
    final O(B) reduction of the per-row scalars; all O(B^2 D) and
    O(B^2) work runs on the NeuronCores.

Outputs per core: S [128,4] f32, l1 [1,512] f32  ->  host combines to
(total_loss, contrastive_dist_loss, normalized_magnitude_loss).
"""

import numpy as np
import ml_dtypes

BF16 = ml_dtypes.bfloat16

B = 4096
D = 768
NCORES = 8
BL = B // NCORES          # 512 rows per core
P = 128                   # partitions
NK = D // P               # 6 full contraction chunks
KEXT = 4                  # hi/lo tsq + hi/lo psq rank-1 rows
NT = BL // P              # 4 m-tiles per core
NJ = B // 512             # 8 n-chunks of 512
PTW = BL + 16             # pt_ext width: 512 cols + 8 f32 bias slots
TTW = [512, 512, 1024, 1024, 1024]   # tt packed column block widths
TTOFF = [0, 512, 1024, 2048, 3072]   # their column offsets
NSCOL = 11                # softmax partial-sum columns (one per ACT chain)
C_STAB = 40.0             # stabilization constant; see module docstring

_COMPILED = None          # cached (nc) bass program
LAST_RESULTS = None       # BassKernelResults of the most recent run


def _build_bass():
    import concourse.bass as bass
    import concourse.mybir as mybir
    import concourse.tile as tile
    import concourse.hw_specs as hw_specs
    from concourse import bacc
    from contextlib import ExitStack

    f32 = mybir.dt.float32
    bf16 = mybir.dt.bfloat16

    # Both Ln and Exp live in the 'natural_log_exp_and_others' ACT table
    # set, but the table-load placement pass resolves each function to the
    # first set containing it (exp_and_others / natural_log), which makes
    # interleaved Ln/Exp reload tables ~14x (~2.7us each). Present those
    # two single-function sets as empty (indices preserved) so both
    # functions resolve to the combined set -> exactly one table load.
    orig_tables = hw_specs.get_activation_tables

    def _tables_one_set(arch):
        t = dict(orig_tables(arch))
        t["exp_and_others"] = set()
        t["natural_log"] = set()
        return t

    hw_specs.get_activation_tables = _tables_one_set
    bacc.get_activation_tables = _tables_one_set
    try:
        return _build_bass_inner(nc_cls=bacc.Bacc)
    finally:
        hw_specs.get_activation_tables = orig_tables
        bacc.get_activation_tables = orig_tables


def _build_bass_inner(nc_cls):
    import concourse.mybir as mybir
    import concourse.tile as tile
    from contextlib import ExitStack

    f32 = mybir.dt.float32
    bf16 = mybir.dt.bfloat16

    nc = nc_cls("TRN2", target_bir_lowering=False, debug=False,
                num_devices=NCORES)

    # pt_ext is widened by 16 bf16 columns: cols 512..519 of the first
    # 128 rows carry the bit pattern of the f32 [128,4] exp-bias vector,
    # so the bias rides inside pt chunk 0's efficient DMA instead of a
    # 128-packets-of-16B transfer of its own (which serializes a queue).
    pt_d = nc.dram_tensor("pt_ext", [D + KEXT, PTW], bf16,
                          kind="ExternalInput").ap()
    # tt arrives pre-packed by the host in column-block-major order
    # (blocks of TTW columns, k-major inside a block), so one DMA per
    # block moves a large contiguous run per partition (high HBM
    # bandwidth) AND delivers K-complete column blocks -- the first
    # softmax chain can start after ~1/12 of the stream.
    ttq_d = nc.dram_tensor("tt_q", [P, NK * B], bf16,
                           kind="ExternalInput").ap()
    tx_d = nc.dram_tensor("tt_x", [KEXT, B], bf16,
                          kind="ExternalInput").ap()
    ts_d = nc.dram_tensor("ts_ext", [D, BL], bf16,
                          kind="ExternalInput").ap()
    s_d = nc.dram_tensor("s_out", [P, NSCOL], f32,
                         kind="ExternalOutput").ap()
    # per-(contraction-partition) |p-t| sums; the final 128-way add is
    # part of the host-side scalar reduction
    l1_d = nc.dram_tensor("l1_out", [P, BL], f32,
                          kind="ExternalOutput").ap()

    with tile.TileContext(nc) as tc, ExitStack() as ctx:
        const_pool = ctx.enter_context(tc.tile_pool(name="consts", bufs=1))
        work_pool = ctx.enter_context(tc.tile_pool(name="work", bufs=2))
        big_pool = ctx.enter_context(tc.tile_pool(name="big", bufs=2))

        HB = B // 2           # 2048: column half processed per ACT step

        # ---- input loads ----
        # One queue at full bandwidth, ordered by when each tensor is
        # first needed: tt quarter 0 + pt chunk 0 + ext rows unblock the
        # first matmul sweep, quarter 1 the second chain, and so on.
        tt_all = const_pool.tile([P, NK * B], bf16, name="tt_all")
        tt3 = tt_all.rearrange("p (k n) -> p k n", k=NK)
        pt_sb = [const_pool.tile([P, PTW], bf16, name=f"pt{k}")
                 for k in range(NK)]
        pt_sb.append(const_pool.tile([KEXT, PTW], bf16, name="pt6"))
        bias_sb = pt_sb[0][:, BL:BL + 8].bitcast(f32)   # [128, 4] f32
        tx_sb = const_pool.tile([KEXT, B], bf16, name="tx_sb")
        ts_sb = [const_pool.tile([P, BL], bf16, name=f"ts{k}")
                 for k in range(NK)]

        def dma_q(b):
            off, w = TTOFF[b], TTW[b]
            nc.sync.dma_start(tt3[:, :, off:off + w],
                              ttq_d[:, NK * off:NK * (off + w)])

        dma_q(0)
        for k in range(NK):
            nc.sync.dma_start(pt_sb[k], pt_d[k * P:(k + 1) * P, :])
        nc.sync.dma_start(pt_sb[NK], pt_d[D:D + KEXT, :])
        nc.sync.dma_start(tx_sb, tx_d)
        for b in range(1, len(TTW)):
            dma_q(b)
        for k in range(NK):
            nc.sync.dma_start(ts_sb[k], ts_d[k * P:(k + 1) * P, :])

        warm_sb = const_pool.tile([P, P], bf16, name="warm_sb")
        nc.gpsimd.memset(warm_sb, 0.0)

        s_sb = const_pool.tile([P, NSCOL], f32, name="s_sb")

        def rhs_cols(k, c0, c1):
            # columns [c0, c1) of contraction chunk k
            if k == NK:
                return tx_sb[:, c0:c1]
            return tt_all[:, k * B + c0:k * B + c1]

        # ---- magnitude loss: l1[m] = sum_d |p - t|, entirely off the
        # critical engines: |diff| and the chunk accumulation run on the
        # (otherwise idle) VectorE, the partition reduction on GpSimd.
        acc = None
        for k in range(NK):
            diff = work_pool.tile([P, BL], bf16, name="diff", tag="diff")
            nc.vector.tensor_tensor(diff, pt_sb[k][:, :BL], ts_sb[k],
                                    op=mybir.AluOpType.subtract)
            ndiff = work_pool.tile([P, BL], bf16, name="ndiff", tag="ndiff")
            nc.vector.tensor_scalar(ndiff, diff, -1.0, None,
                                    op0=mybir.AluOpType.mult)
            absd = work_pool.tile([P, BL], f32, name="absd", tag="absd",
                                  bufs=3)
            nc.vector.tensor_tensor(absd, diff, ndiff,
                                    op=mybir.AluOpType.max)
            if acc is None:
                acc = absd
            else:
                nacc = work_pool.tile([P, BL], f32, name="nacc", tag="absd",
                                      bufs=3)
                nc.vector.tensor_tensor(nacc, acc, absd,
                                        op=mybir.AluOpType.add)
                acc = nacc
        nc.sync.dma_start(l1_d, acc)

        # ---- main: X = -d^2/2 on PE; d = exp(.5 ln(-2X)); softmax sums ----
        # Column-half-major order (all m-tiles' half 0, then half 1) so
        # the whole first phase only needs tt quarters 0-1.  Per chain:
        # k-outer matmul sweep -> Ln (PSUM drain) -> exp(.5*) ->
        # exp(-10*+bias) with fused row-accumulation.
        def act_chain(xq_slice, t, cols, s_col):
            w = cols.stop - cols.start
            lnq = big_pool.tile([P, w], f32, name="lnq", tag="lnq")
            nc.scalar.activation(lnq, xq_slice,
                                 mybir.ActivationFunctionType.Ln,
                                 scale=-2.0)
            dmat = big_pool.tile([P, w], f32, name="dmat", tag="dmat")
            nc.scalar.activation(dmat, lnq,
                                 mybir.ActivationFunctionType.Exp,
                                 scale=0.5)
            emat = big_pool.tile([P, w], f32, name="emat", tag="emat")
            nc.scalar.activation(emat, dmat,
                                 mybir.ActivationFunctionType.Exp,
                                 scale=-10.0,
                                 bias=bias_sb[:, t:t + 1],
                                 accum_out=s_sb[:, s_col:s_col + 1])

        s_col = 0
        with tc.tile_pool(name="psum_x", bufs=2, space="PSUM") as psum_x:
            # PE HAM warm-up: dense N=128 matmuls on a zero tile so the
            # clock gate opens (1.2 -> 2.4 GHz) right as the first tt
            # block lands; they only depend on a memset and release their
            # PSUM slot immediately.
            warm_ps = psum_x.tile([P, P], f32, name="warm_ps", tag="xq")
            for _ in range(40):
                nc.tensor.matmul(warm_ps, lhsT=warm_sb, rhs=warm_sb,
                                 start=True, stop=True)
            for h in range(2):
                for t in range(NT):
                    xq = psum_x.tile([P, HB], f32, name="xq", tag="xq")
                    # the first m-tile-half's chains follow the packed
                    # tt block widths (ScalarE starts right after block 0
                    # lands); the last is split to shorten the tail
                    if h == 0 and t == 0:
                        widths = [512, 512, 1024]
                    elif h == 1 and t == NT - 1:
                        widths = [1024, 1024]
                    else:
                        widths = [HB]
                    o = 0
                    for sw in widths:
                        c0 = h * HB + o
                        for k in range(NK + 1):
                            for jl in range(sw // 512):
                                nc.tensor.matmul(
                                    xq[:, o + jl * 512:
                                       o + (jl + 1) * 512],
                                    lhsT=pt_sb[k][:, t * P:(t + 1) * P],
                                    rhs=rhs_cols(k, c0 + jl * 512,
                                                 c0 + (jl + 1) * 512),
                                    start=(k == 0), stop=(k == NK))
                        act_chain(xq[:, o:o + sw], t,
                                  slice(c0, c0 + sw), s_col)
                        s_col += 1
                        o += sw
            nc.sync.dma_start(s_d, s_sb)

    nc.compile()
    return nc


def _get_compiled():
    global _COMPILED
    if _COMPILED is None:
        _COMPILED = _build_bass()
    return _COMPILED


def _split_bf16(v):
    hi = v.astype(np.float32).astype(BF16)
    lo = (v.astype(np.float32) - hi.astype(np.float32)).astype(BF16)
    return hi, lo


def kernel(predicted, target):
    global LAST_RESULTS
    from concourse.bass_utils import run_bass_kernel_spmd

    p = np.ascontiguousarray(np.asarray(predicted, dtype=np.float32))
    t = np.ascontiguousarray(np.asarray(target, dtype=np.float32))
    assert p.shape == (B, D) and t.shape == (B, D)

    # host-side O(B*D) row stats (input prep for the device program)
    p64 = p.astype(np.float64)
    t64 = t.astype(np.float64)
    psq = (p64 * p64).sum(1)
    tsq = (t64 * t64).sum(1)
    tmag = np.abs(t64).sum(1)
    dii = np.sqrt(((p64 - t64) ** 2).sum(1))

    # tt packed column-block-major (see _build_bass_inner)
    ttT = np.ascontiguousarray(t.T).astype(BF16)          # [768, 4096]
    tt6 = ttT.reshape(NK, P, B)
    tt_q = np.concatenate(
        [np.ascontiguousarray(tt6[:, :, off:off + w].transpose(1, 0, 2))
           .reshape(P, NK * w)
         for off, w in zip(TTOFF, TTW)], axis=1)
    tt_q = np.ascontiguousarray(tt_q)
    tt_x = np.zeros((KEXT, B), dtype=BF16)
    hi, lo = _split_bf16(-0.5 * tsq)
    tt_x[0] = hi
    tt_x[1] = lo
    tt_x[2] = BF16(1.0)
    tt_x[3] = BF16(1.0)

    in_maps = []
    for c in range(NCORES):
        sl = slice(c * BL, (c + 1) * BL)
        pt_ext = np.zeros((D + KEXT, PTW), dtype=BF16)
        pt_ext[:D, :BL] = np.ascontiguousarray(p[sl].T).astype(BF16)
        pt_ext[D + 0, :BL] = BF16(1.0)
        pt_ext[D + 1, :BL] = BF16(1.0)
        hi, lo = _split_bf16(-0.5 * psq[sl])
        pt_ext[D + 2, :BL] = hi
        pt_ext[D + 3, :BL] = lo
        # f32 exp-bias vector [128, NT] rides as raw bits in cols 512..519
        # of the first 128 rows (see _build_bass_inner)
        bias = np.ascontiguousarray(
            (10.0 * dii[sl] - C_STAB).astype(np.float32).reshape(NT, P).T)
        pt_ext.view(np.uint16)[:P, BL:BL + 8] = bias.view(np.uint16)
        ts_ext = np.ascontiguousarray(t[sl].T).astype(BF16)
        in_maps.append({
            "pt_ext": pt_ext,
            "tt_q": tt_q,
            "tt_x": tt_x,
            "ts_ext": ts_ext,
        })

    nc = _get_compiled()
    res = run_bass_kernel_spmd(nc, in_maps, core_ids=list(range(NCORES)))
    LAST_RESULTS = res

    S = np.empty(B, dtype=np.float64)
    l1 = np.empty(B, dtype=np.float64)
    for c in range(NCORES):
        out = res.results[c]
        # s_out columns are per-chain partial sums; chains were emitted
        # half-major with (h0,t0) split in three and (h1,t3) in two
        # (cols: t0 -> 0,1,2,6; t1 -> 3,7; t2 -> 4,8; t3 -> 5,9,10).
        s = out["s_out"].astype(np.float64)
        s_full = np.stack([s[:, 0] + s[:, 1] + s[:, 2] + s[:, 6],
                           s[:, 3] + s[:, 7],
                           s[:, 4] + s[:, 8],
                           s[:, 5] + s[:, 9] + s[:, 10]], axis=1)
        S[c * BL:(c + 1) * BL] = s_full.T.reshape(BL)
        l1[c * BL:(c + 1) * BL] = out["l1_out"].astype(np.float64).sum(0)

    contrastive = float(np.log(S).mean() + C_STAB)
    magnitude = float((l1 / tmag).mean())
    total = 0.5 * contrastive + 0.5 * magnitude
    return (np.float32(total), np.float32(contrastive), np.float32(magnitude))


# revision 41
# speedup vs baseline: 1.0371x; 1.0371x over previous
"""ContrastiveMagnitudeLoss on 8 Trainium2 NeuronCores (Bass/Tile).

Strategy (sharding_hint: shard batch across cores, all-gather target):
  - B=4096 rows of `predicted` are sharded 512/core. Every core gets the
    full (transposed) `target`, so each core owns complete rows of the
    B x B distance matrix and the row-softmax needs no communication.
  - The Gram identity  d^2[m,n] = ||p_m||^2 + ||t_n||^2 - 2 p_m.t_n  is
    computed entirely on the PE array by extending the contraction dim:
    4 extra K-rows carry (1, -tsq/2) and (-psq/2, 1) rank-1 terms (each
    split hi/lo in bf16 to keep f32-level accuracy), so PSUM directly
    holds X = -d^2/2.
  - ScalarE evaluates d = exp(0.5*ln(-2X)) (Ln+Exp share one ACT table
    set; Sqrt would force table thrashing and has a loose ULP budget),
    then exp(-10*d + b_i) with per-row bias b_i = 10*d_ii - 40 and a
    fused free-dim accumulation (accum_out) giving the softmax sums S_i.
    Algebra: logsumexp_i - logit_ii == ln(S_i) + 40 exactly, so only
    S_i [B] leaves the device for the contrastive term.
  - The magnitude-loss numerator sum_d |p - t| is reduced over the
    contraction dim with a ones-vector matmul on PE.
  - Host does the O(B*D) input prep (transpose/shard/row stats) and the
    final O(B) reduction of the per-row scalars; all O(B^2 D) and
    O(B^2) work runs on the NeuronCores.

Outputs per core: S [128,4] f32, l1 [1,512] f32  ->  host combines to
(total_loss, contrastive_dist_loss, normalized_magnitude_loss).
"""

import numpy as np
import ml_dtypes

BF16 = ml_dtypes.bfloat16

B = 4096
D = 768
NCORES = 8
BL = B // NCORES          # 512 rows per core
P = 128                   # partitions
NK = D // P               # 6 full contraction chunks
KEXT = 4                  # hi/lo tsq + hi/lo psq rank-1 rows
NT = BL // P              # 4 m-tiles per core
NJ = B // 512             # 8 n-chunks of 512
PTW = BL + 16             # pt_ext width: 512 cols + 8 f32 bias slots
TTW = [512, 512, 1024, 1024, 1024]   # tt packed column block widths
TTOFF = [0, 512, 1024, 2048, 3072]   # their column offsets
NSCOL = 11                # softmax partial-sum columns (one per ACT chain)
C_STAB = 40.0             # stabilization constant; see module docstring

_COMPILED = None          # cached (nc) bass program
LAST_RESULTS = None       # BassKernelResults of the most recent run


def _build_bass():
    import concourse.bass as bass
    import concourse.mybir as mybir
    import concourse.tile as tile
    import concourse.hw_specs as hw_specs
    from concourse import bacc
    from contextlib import ExitStack

    f32 = mybir.dt.float32
    bf16 = mybir.dt.bfloat16

    # Both Ln and Exp live in the 'natural_log_exp_and_others' ACT table
    # set, but the table-load placement pass resolves each function to the
    # first set containing it (exp_and_others / natural_log), which makes
    # interleaved Ln/Exp reload tables ~14x (~2.7us each). Present those
    # two single-function sets as empty (indices preserved) so both
    # functions resolve to the combined set -> exactly one table load.
    orig_tables = hw_specs.get_activation_tables

    def _tables_one_set(arch):
        t = dict(orig_tables(arch))
        t["exp_and_others"] = set()
        t["natural_log"] = set()
        return t

    hw_specs.get_activation_tables = _tables_one_set
    bacc.get_activation_tables = _tables_one_set
    try:
        return _build_bass_inner(nc_cls=bacc.Bacc)
    finally:
        hw_specs.get_activation_tables = orig_tables
        bacc.get_activation_tables = orig_tables


def _build_bass_inner(nc_cls):
    import concourse.mybir as mybir
    import concourse.tile as tile
    from contextlib import ExitStack

    f32 = mybir.dt.float32
    bf16 = mybir.dt.bfloat16

    nc = nc_cls("TRN2", target_bir_lowering=False, debug=False,
                num_devices=NCORES)

    # pt_ext is widened by 16 bf16 columns: cols 512..519 of the first
    # 128 rows carry the bit pattern of the f32 [128,4] exp-bias vector,
    # so the bias rides inside pt chunk 0's efficient DMA instead of a
    # 128-packets-of-16B transfer of its own (which serializes a queue).
    # pt is packed k-major like tt: pt_pk[p, k*PTW + c] = chunk k row p,
    # one DMA with 7.4 KB contiguous per partition; chunk 6 holds the
    # KEXT ext rows on partitions 0..3 (zeros elsewhere)
    pt_d = nc.dram_tensor("pt_pk", [P, (NK + 1) * PTW], bf16,
                          kind="ExternalInput").ap()
    # tt arrives pre-packed by the host in column-block-major order
    # (blocks of TTW columns, k-major inside a block), so one DMA per
    # block moves a large contiguous run per partition (high HBM
    # bandwidth) AND delivers K-complete column blocks -- the first
    # softmax chain can start after ~1/12 of the stream.
    ttq_d = nc.dram_tensor("tt_q", [P, NK * B], bf16,
                           kind="ExternalInput").ap()
    tx_d = nc.dram_tensor("tt_x", [KEXT, B], bf16,
                          kind="ExternalInput").ap()
    ts_d = nc.dram_tensor("ts_ext", [D, BL], bf16,
                          kind="ExternalInput").ap()
    s_d = nc.dram_tensor("s_out", [P, NSCOL], f32,
                         kind="ExternalOutput").ap()
    # per-(contraction-partition) |p-t| sums; the final 128-way add is
    # part of the host-side scalar reduction
    l1_d = nc.dram_tensor("l1_out", [P, BL], f32,
                          kind="ExternalOutput").ap()

    with tile.TileContext(nc) as tc, ExitStack() as ctx:
        const_pool = ctx.enter_context(tc.tile_pool(name="consts", bufs=1))
        work_pool = ctx.enter_context(tc.tile_pool(name="work", bufs=2))
        big_pool = ctx.enter_context(tc.tile_pool(name="big", bufs=2))

        HB = B // 2           # 2048: column half processed per ACT step

        # ---- input loads ----
        # One queue at full bandwidth, ordered by when each tensor is
        # first needed: tt quarter 0 + pt chunk 0 + ext rows unblock the
        # first matmul sweep, quarter 1 the second chain, and so on.
        tt_all = const_pool.tile([P, NK * B], bf16, name="tt_all")
        tt3 = tt_all.rearrange("p (k n) -> p k n", k=NK)
        pt_all = const_pool.tile([P, (NK + 1) * PTW], bf16, name="pt_all")
        bias_sb = pt_all[:, BL:BL + 8].bitcast(f32)     # [128, 4] f32
        tx_sb = const_pool.tile([KEXT, B], bf16, name="tx_sb")
        ts_sb = [const_pool.tile([P, BL], bf16, name=f"ts{k}")
                 for k in range(NK)]

        def dma_q(b):
            off, w = TTOFF[b], TTW[b]
            nc.sync.dma_start(tt3[:, :, off:off + w],
                              ttq_d[:, NK * off:NK * (off + w)])

        dma_q(0)
        nc.sync.dma_start(pt_all, pt_d)
        nc.sync.dma_start(tx_sb, tx_d)
        for b in range(1, len(TTW)):
            dma_q(b)
        for k in range(NK):
            nc.sync.dma_start(ts_sb[k], ts_d[k * P:(k + 1) * P, :])

        warm_sb = const_pool.tile([P, P], bf16, name="warm_sb")
        nc.gpsimd.memset(warm_sb, 0.0)

        s_sb = const_pool.tile([P, NSCOL], f32, name="s_sb")

        def pt_lhs(k, t):
            base = k * PTW + t * P
            if k == NK:
                return pt_all[0:KEXT, base:base + P]
            return pt_all[:, base:base + P]

        def rhs_cols(k, c0, c1):
            # columns [c0, c1) of contraction chunk k
            if k == NK:
                return tx_sb[:, c0:c1]
            return tt_all[:, k * B + c0:k * B + c1]

        # ---- magnitude loss: l1[m] = sum_d |p - t|, entirely off the
        # critical engines: |diff| and the chunk accumulation run on the
        # (otherwise idle) VectorE, the partition reduction on GpSimd.
        acc = None
        for k in range(NK):
            diff = work_pool.tile([P, BL], bf16, name="diff", tag="diff")
            nc.vector.tensor_tensor(diff, pt_all[:, k * PTW:k * PTW + BL],
                                    ts_sb[k], op=mybir.AluOpType.subtract)
            ndiff = work_pool.tile([P, BL], bf16, name="ndiff", tag="ndiff")
            nc.vector.tensor_scalar(ndiff, diff, -1.0, None,
                                    op0=mybir.AluOpType.mult)
            absd = work_pool.tile([P, BL], f32, name="absd", tag="absd",
                                  bufs=3)
            nc.vector.tensor_tensor(absd, diff, ndiff,
                                    op=mybir.AluOpType.max)
            if acc is None:
                acc = absd
            else:
                nacc = work_pool.tile([P, BL], f32, name="nacc", tag="absd",
                                      bufs=3)
                nc.vector.tensor_tensor(nacc, acc, absd,
                                        op=mybir.AluOpType.add)
                acc = nacc
        nc.sync.dma_start(l1_d, acc)

        # ---- main: X = -d^2/2 on PE; d = exp(.5 ln(-2X)); softmax sums ----
        # Column-half-major order (all m-tiles' half 0, then half 1) so
        # the whole first phase only needs tt quarters 0-1.  Per chain:
        # k-outer matmul sweep -> Ln (PSUM drain) -> exp(.5*) ->
        # exp(-10*+bias) with fused row-accumulation.
        def act_chain(xq_slice, t, cols, s_col):
            w = cols.stop - cols.start
            lnq = big_pool.tile([P, w], f32, name="lnq", tag="lnq")
            nc.scalar.activation(lnq, xq_slice,
                                 mybir.ActivationFunctionType.Ln,
                                 scale=-2.0)
            dmat = big_pool.tile([P, w], f32, name="dmat", tag="dmat")
            nc.scalar.activation(dmat, lnq,
                                 mybir.ActivationFunctionType.Exp,
                                 scale=0.5)
            emat = big_pool.tile([P, w], f32, name="emat", tag="emat")
            nc.scalar.activation(emat, dmat,
                                 mybir.ActivationFunctionType.Exp,
                                 scale=-10.0,
                                 bias=bias_sb[:, t:t + 1],
                                 accum_out=s_sb[:, s_col:s_col + 1])

        s_col = 0
        with tc.tile_pool(name="psum_x", bufs=2, space="PSUM") as psum_x:
            # PE HAM warm-up: dense N=128 matmuls on a zero tile so the
            # clock gate opens (1.2 -> 2.4 GHz) right as the first tt
            # block lands; they only depend on a memset and release their
            # PSUM slot immediately.
            warm_ps = psum_x.tile([P, P], f32, name="warm_ps", tag="xq")
            for _ in range(40):
                nc.tensor.matmul(warm_ps, lhsT=warm_sb, rhs=warm_sb,
                                 start=True, stop=True)
            for h in range(2):
                for t in range(NT):
                    xq = psum_x.tile([P, HB], f32, name="xq", tag="xq")
                    # the first m-tile-half's chains follow the packed
                    # tt block widths (ScalarE starts right after block 0
                    # lands); the last is split to shorten the tail
                    if h == 0 and t == 0:
                        widths = [512, 512, 1024]
                    elif h == 1 and t == NT - 1:
                        widths = [1024, 1024]
                    else:
                        widths = [HB]
                    o = 0
                    for sw in widths:
                        c0 = h * HB + o
                        for k in range(NK + 1):
                            for jl in range(sw // 512):
                                nc.tensor.matmul(
                                    xq[:, o + jl * 512:
                                       o + (jl + 1) * 512],
                                    lhsT=pt_lhs(k, t),
                                    rhs=rhs_cols(k, c0 + jl * 512,
                                                 c0 + (jl + 1) * 512),
                                    start=(k == 0), stop=(k == NK))
                        act_chain(xq[:, o:o + sw], t,
                                  slice(c0, c0 + sw), s_col)
                        s_col += 1
                        o += sw
            nc.sync.dma_start(s_d, s_sb)

    nc.compile()
    return nc


def _get_compiled():
    global _COMPILED
    if _COMPILED is None:
        _COMPILED = _build_bass()
    return _COMPILED


def _split_bf16(v):
    hi = v.astype(np.float32).astype(BF16)
    lo = (v.astype(np.float32) - hi.astype(np.float32)).astype(BF16)
    return hi, lo


def kernel(predicted, target):
    global LAST_RESULTS
    from concourse.bass_utils import run_bass_kernel_spmd

    p = np.ascontiguousarray(np.asarray(predicted, dtype=np.float32))
    t = np.ascontiguousarray(np.asarray(target, dtype=np.float32))
    assert p.shape == (B, D) and t.shape == (B, D)

    # host-side O(B*D) row stats (input prep for the device program)
    p64 = p.astype(np.float64)
    t64 = t.astype(np.float64)
    psq = (p64 * p64).sum(1)
    tsq = (t64 * t64).sum(1)
    tmag = np.abs(t64).sum(1)
    dii = np.sqrt(((p64 - t64) ** 2).sum(1))

    # tt packed column-block-major (see _build_bass_inner)
    ttT = np.ascontiguousarray(t.T).astype(BF16)          # [768, 4096]
    tt6 = ttT.reshape(NK, P, B)
    tt_q = np.concatenate(
        [np.ascontiguousarray(tt6[:, :, off:off + w].transpose(1, 0, 2))
           .reshape(P, NK * w)
         for off, w in zip(TTOFF, TTW)], axis=1)
    tt_q = np.ascontiguousarray(tt_q)
    tt_x = np.zeros((KEXT, B), dtype=BF16)
    hi, lo = _split_bf16(-0.5 * tsq)
    tt_x[0] = hi
    tt_x[1] = lo
    tt_x[2] = BF16(1.0)
    tt_x[3] = BF16(1.0)

    in_maps = []
    for c in range(NCORES):
        sl = slice(c * BL, (c + 1) * BL)
        pt_ext = np.zeros((NK + 1, P, PTW), dtype=BF16)
        pt_ext[:NK, :, :BL] = (
            np.ascontiguousarray(p[sl].T).astype(BF16).reshape(NK, P, BL))
        pt_ext[NK, 0, :BL] = BF16(1.0)
        pt_ext[NK, 1, :BL] = BF16(1.0)
        hi, lo = _split_bf16(-0.5 * psq[sl])
        pt_ext[NK, 2, :BL] = hi
        pt_ext[NK, 3, :BL] = lo
        # f32 exp-bias vector [128, NT] rides as raw bits in cols 512..519
        # of chunk 0 (see _build_bass_inner)
        bias = np.ascontiguousarray(
            (10.0 * dii[sl] - C_STAB).astype(np.float32).reshape(NT, P).T)
        pt_ext[0].view(np.uint16)[:, BL:BL + 8] = bias.view(np.uint16)
        pt_ext = np.ascontiguousarray(
            pt_ext.transpose(1, 0, 2).reshape(P, (NK + 1) * PTW))
        ts_ext = np.ascontiguousarray(t[sl].T).astype(BF16)
        in_maps.append({
            "pt_pk": pt_ext,
            "tt_q": tt_q,
            "tt_x": tt_x,
            "ts_ext": ts_ext,
        })

    nc = _get_compiled()
    res = run_bass_kernel_spmd(nc, in_maps, core_ids=list(range(NCORES)))
    LAST_RESULTS = res

    S = np.empty(B, dtype=np.float64)
    l1 = np.empty(B, dtype=np.float64)
    for c in range(NCORES):
        out = res.results[c]
        # s_out columns are per-chain partial sums; chains were emitted
        # half-major with (h0,t0) split in three and (h1,t3) in two
        # (cols: t0 -> 0,1,2,6; t1 -> 3,7; t2 -> 4,8; t3 -> 5,9,10).
        s = out["s_out"].astype(np.float64)
        s_full = np.stack([s[:, 0] + s[:, 1] + s[:, 2] + s[:, 6],
                           s[:, 3] + s[:, 7],
                           s[:, 4] + s[:, 8],
                           s[:, 5] + s[:, 9] + s[:, 10]], axis=1)
        S[c * BL:(c + 1) * BL] = s_full.T.reshape(BL)
        l1[c * BL:(c + 1) * BL] = out["l1_out"].astype(np.float64).sum(0)

    contrastive = float(np.log(S).mean() + C_STAB)
    magnitude = float((l1 / tmag).mean())
    total = 0.5 * contrastive + 0.5 * magnitude
    return (np.float32(total), np.float32(contrastive), np.float32(magnitude))
